# revision 67
# baseline (speedup 1.0000x reference)
"""Trainium2 Bass kernel for nn_DeepSetClassifier (deep-set pooling + gelu MLP).

Math (per batch b, expert e, row i, col j, hidden d; N=128, DIM=32):
    rowsum[i] = sum_j mask[i,j];  denom = max(rowsum, 1);  rinv = 1/denom
    zm[e,i]   = sum_j mask[i,j] * z[e,i,j]
    a[e,i] = zm*rinv ; r[i] = rowsum*rinv
    beta[e,i,d] = wself_b[d] + u[d]*a[e,i] + v[d]*r[i]     (u = wctx@phi_w, v = wctx@phi_b)
    out[e,i,j] = out_b + sum_d out_w[d] * gelu(wself_w[d]*z[e,i,j] + beta[e,i,d])

Sharding/dispatch: ALL work on ONE core, as N_SPLIT=4 pipelined async
dispatches of N_B=2 batches each. One core because the axon tunnel
charges a large per-device fan-out penalty (a 32-byte 8-way sharded
device_put costs ~85 ms; the same 1.25 MiB to one device costs ~72 ms)
while the extra on-chip work is <1 ms. Pipelined because part p's
upload overlaps part p-1's execute+download on the full-duplex wire,
and each part is packed right before its dispatch so host pack time
overlaps the wire too. Interleaved: 1 dispatch 93.5/95.2 ms (min/p25),
2x4-batch 83.6/89.0, 4x2-batch 77.2/84.0, 8x1-batch 85.6/90.1.

Engine plan per dispatch (N_B batches x 4 "pairs"; a pair = 2 e values):
  - DVE+GPSIMD: build IN[e][i,(d,j)] = z*s_d + beta_d
    (GPSIMD: fused tensor_scalar with two AP scalars — verified exact on HW.
     DVE: scalar_tensor_tensor with one AP scalar + broadcast tensor.)
  - ACT: one big gelu per pair over [128, 32*128]
  - PE: reduce over d via 32 accumulating matmuls with diagonal stationary
    w_d*I (float32r, moving N=256 = 2 pairs) into PSUM
  - DVE: PSUM + out_b -> fp16 SBUF (ulp << int8 step); after all pairs:
    global absmax (DVE max/min reduces + gpsimd partition_all_reduce),
    quantize with ONE fused ACT pass (Copy with scale=127/absmax; the
    ACT int8 output conversion rounds to nearest even — probed),
    DMA out + scale bytes

Dispatch plan (dominant cost — the 8 cores sit behind an axon tunnel with
~40-85 ms RTT, highly variable latency, and upstream-expensive wire):
  - The jitted shard_map executable is built ONCE per process and cached;
    re-running run_bass_kernel_spmd per call re-traces, re-lowers and
    re-loads the NEFF (~500 ms/call).
  - AOT-compiled with the bass effect suppressed (C++ fast-path dispatch).
  - sdiag (the 2 MiB/core PE stationary w_d*I) is built on-chip with one
    gpsimd affine_select over an iota predicate instead of shipped (16 MiB).
  - No donated zero output buffers: outputs are plain custom-call results
    (the kernel writes every element).
  - Wire format up, per part: ONE tightly packed [2182,128] int8 tensor
    (273 KiB): rows 0..2047 = the part's 16 z matrices quantized int8
    (one absmax scale per part, folded into the host-side consts:
    wself_w and u pre-multiplied, so the on-chip math never sees the
    scale); rows 2048..2175 = the part's masks BITPACKED (row i, cols
    [16b,16b+16) = packbits(mask[b,i,:]); unpacked on-chip with DVE
    shift+and, probed exact); rows 2176.. = the 161 f32 consts as raw
    bytes (bitcast back to f32 on-chip).
  - Wire format down, per part: ONE [2049,128] int8 tensor (256 KiB):
    rows 0..2047 = out int8 (scale = on-chip absmax via DVE max/min
    reduces + gpsimd partition_all_reduce), row 2048 carries the f32
    scale bytes. Dequantized on host. Combined z-int8 + out-int8 + fp16
    out staging rel err 1.09% (measured = simulated) vs the 2e-2 gate.
  - ONE input tensor and ONE output tensor per part, passed as numpy
    args to the AOT-compiled fn: every extra put call serializes an
    additional protocol phase on the tunnel (measured +60 ms), and every
    un-prefetched output fetch costs a full extra RTT (measured +80 ms)
    — hence copy_to_host_async on every output right at dispatch.
  - Net: ~77-95 ms/call at a ~72-85 ms pure-RTT floor (tunnel drifts;
    the 8-core fp16 predecessor measured 94-148 ms, same-window 148->85).
"""

import numpy as np

import jax
import jax.numpy as jnp
from jax.experimental.shard_map import shard_map
from jax.sharding import Mesh, NamedSharding, PartitionSpec

import concourse.bass as bass
import concourse.bacc as bacc
import concourse.tile as tile
from concourse import mybir
from concourse import bass2jax as b2j
from concourse.bass_isa import ReduceOp

F32 = mybir.dt.float32
F32R = mybir.dt.float32r
HALF = mybir.dt.float16
U8 = mybir.dt.uint8
I8 = mybir.dt.int8
AX = mybir.AxisListType
OP = mybir.AluOpType
AF = mybir.ActivationFunctionType

E, N, DIM = 8, 128, 32
NCORES = 8
# All work runs on ONE core: the axon tunnel charges a large per-device
# fan-out penalty (a 32-byte 8-way sharded device_put costs ~85 ms; the
# SAME 1.25 MiB to one device costs ~72 ms), while the extra on-chip
# work is <1 ms. Measured min 96.4 (1 core) vs 102.7 ms (8 cores).
# The call is further PIPELINED as N_SPLIT sequential dispatches of N_B
# batches each on that core: part p's upload overlaps part p-1's
# execute+download on the full-duplex tunnel, and each part is packed
# right before its dispatch so host pack time overlaps the wire too.
# Interleaved: full 93.5/95.2 (min/p25), 2x4-batch 83.6/89.0,
# 4x2-batch 77.2/84.0, 8x1-batch 85.6/90.1 -> 4 parts of 2 batches.
N_B = 2
N_SPLIT = 4
NCORES_USED = 1

# consts layout (columns of the [1, CC] consts input; broadcast down
# partitions on-chip): wself_w | u | v | wself_b | out_b | out_w
C_S = 0
C_U = DIM
C_V = 2 * DIM
C_WSB = 3 * DIM
C_OB = 4 * DIM
C_OW = 4 * DIM + 1
CC = 5 * DIM + 1

PE_DTYPE = F32R
N_DVE_DS = 16

def _bcast_col(col_ap, n):
    """[128,1] column AP -> [128,n] stride-0 broadcast along free dim."""
    return bass.AP(tensor=col_ap.tensor, offset=col_ap.offset,
                   ap=[col_ap.ap[0], [0, n]])


def _ow_diag_src(consts, n):
    """AP reading consts[i, C_OW+d] at logical index [i, d, j] (j bcast)."""
    base = consts[:, C_OW:C_OW + DIM]
    return bass.AP(tensor=base.tensor, offset=base.offset,
                   ap=[base.ap[0], list(base.ap[1]), [0, n]])


def build_bass(ncores=None, n_e=E, n_b=None):
    """n_b = batches handled by EACH core (1 = classic 8-core data
    parallel; 8 = the whole problem on one core, which avoids the
    per-device fan-out penalty of the axon tunnel)."""
    pe_dt = PE_DTYPE
    if n_b is None:
        n_b = N_B
    nc = bacc.Bacc("TRN2", target_bir_lowering=False, debug=False,
                   num_devices=ncores or (NCORES // n_b))
    nt = n_b * n_e

    # ONE input tensor, tightly packed 2D [R, N] int8:
    #   rows [t*N,(t+1)*N) for t=b*n_e+e : z[b,e] quantized int8 (absmax
    #     scale over the core's batches, folded into the host-side consts)
    #   next n_b*16 rows : mask BITS, linear layout — partition i's bits
    #     for all n_b batches at byte offset i*(n_b*16), batch b at
    #     [16b,16b+16), MSB-first packbits(mask[b,i,:]) (2 KiB/batch
    #     instead of 16)
    #   last rows : the CC f32 consts as raw bytes (bitcast on read).
    # One tensor = one transfer per dispatch on the tunnel.
    mrows = n_b * 16
    zrows = nt * N + mrows + (CC * 4 + N - 1) // N
    z_dram = nc.dram_tensor("z", [zrows, N], I8, kind="ExternalInput")
    # ONE output tensor: rows [(b*n_e+e)*N + i] = out int8; row [nt*N]
    # bytes 0..3 = the on-chip-computed absmax scale as raw f32 bytes.
    # The f32->int8 ACT output conversion rounds to nearest even (probed).
    out_dram = nc.dram_tensor("out", [nt * N + 1, N], I8,
                              kind="ExternalOutput")

    dve_ds = tuple(range(N_DVE_DS))

    with tile.TileContext(nc) as tc:
        with (
            tc.tile_pool(name="singles", bufs=1) as singles,
            tc.tile_pool(name="zpool", bufs=4) as zpool,
            tc.tile_pool(name="small", bufs=4) as small,
            tc.tile_pool(name="mpool", bufs=2) as mpool,
            tc.tile_pool(name="inpool", bufs=3) as inpool,
            tc.tile_pool(name="gpool", bufs=2) as gpool,
            tc.tile_pool(name="psum", bufs=3, space="PSUM") as psump,
        ):
            # bcast [1,CC] consts row down 128 partitions: ones^T @ row
            crow = singles.tile([1, CC], F32)
            c_src = bass.AP(tensor=z_dram[0:1, 0:1].tensor,
                            offset=(nt * N + mrows) * N,
                            ap=[[4, 1], [1, CC * 4]]).bitcast(F32)
            nc.sync.dma_start(out=crow, in_=c_src)

            # ONE DMA pulls every batch's mask bits: partition i <- the
            # n_b*16 bytes at linear offset i*(n_b*16) of the bits block
            mball = singles.tile([N, mrows], U8)
            nc.sync.dma_start(
                out=mball,
                in_=bass.AP(tensor=z_dram[0:1, 0:1].tensor,
                            offset=nt * N * N,
                            ap=[[mrows, N], [1, mrows]]).bitcast(U8))
            ones = singles.tile([1, N], F32)
            nc.gpsimd.memset(ones, 1.0)
            ps_c = psump.tile([N, CC], F32, tag="cbcast")
            nc.tensor.matmul(out=ps_c, lhsT=ones, rhs=crow,
                             start=True, stop=True)
            consts = singles.tile([N, CC], F32)
            nc.scalar.copy(out=consts, in_=ps_c)

            s_cols = consts[:, C_S:C_S + DIM]
            u_cols = consts[:, C_U:C_U + DIM]
            v_cols = consts[:, C_V:C_V + DIM]
            wsb_cols = consts[:, C_WSB:C_WSB + DIM]
            ob_col = consts[:, C_OB:C_OB + 1]

            # sd[i, d, j] = out_w[d] * (i == j) — PE stationary, built on-chip
            sd = singles.tile([N, DIM, N], pe_dt)
            nc.gpsimd.affine_select(
                out=sd[:, :, :], in_=_ow_diag_src(consts, N),
                pattern=[[0, DIM], [-1, N]], compare_op=OP.is_equal,
                fill=0.0, base=0, channel_multiplier=1)

            # all outputs stay on-chip (fp16: ulp << the int8 step) until
            # the global absmax is known, then quantize in one ACT pass
            oall = singles.tile([N, nt * N], HALF)

            for b in range(n_b):
                # --- mask pooling prep (once per batch) ---
                # unpack mask bits: msk[i, 8k+t] = (byte[i,k] >> (7-t)) & 1
                mb8 = mball[:, 16 * b:16 * b + 16]
                mu8 = mpool.tile([N, N], U8, tag="mu8")
                for t in range(8):
                    view = bass.AP(tensor=mu8.tensor, offset=mu8.offset + t,
                                   ap=[mu8.ap[0], [8, 16]])
                    nc.vector.tensor_scalar(
                        out=view, in0=mb8, scalar1=7 - t, scalar2=1,
                        op0=OP.logical_shift_right, op1=OP.bitwise_and)
                msk = mpool.tile([N, N], F32, tag="m")
                nc.scalar.copy(out=msk, in_=mu8)
                rowsum = small.tile([N, 1], F32, tag="rowsum")
                nc.vector.tensor_reduce(out=rowsum, in_=msk, axis=AX.X,
                                        op=OP.add)
                denom = small.tile([N, 1], F32, tag="denom")
                nc.vector.tensor_scalar_max(denom, rowsum, 1.0)
                rinv = small.tile([N, 1], F32, tag="rinv")
                nc.vector.reciprocal(out=rinv, in_=denom)
                rr = small.tile([N, 1], F32, tag="rr")
                nc.vector.tensor_mul(rr, rowsum, rinv)
                # W0[i,d] = wself_b[d] + v[d]*r[i] (gpsimd fused 2-op ok)
                w0 = small.tile([N, DIM], F32, tag="w0")
                nc.gpsimd.tensor_scalar(out=w0, in0=v_cols, scalar1=rr,
                                        scalar2=None, op0=OP.mult)
                nc.vector.tensor_add(w0, w0, wsb_cols)

                for g in range(n_e // 2):
                    gtile = gpool.tile([N, DIM, 2, N], pe_dt, tag="g2")
                    for k in range(2):
                        e = 2 * g + k
                        t0z = (b * n_e + e) * N
                        ze_raw = zpool.tile([N, N], I8, tag="zraw")
                        nc.sync.dma_start(out=ze_raw,
                                          in_=z_dram[t0z:t0z + N, :])
                        ze = zpool.tile([N, N], F32, tag="z")
                        nc.scalar.copy(out=ze, in_=ze_raw)

                        # zm[i] = sum_j mask*z
                        tmp = zpool.tile([N, N], F32, tag="tmp")
                        nc.vector.tensor_mul(tmp, ze, msk)
                        zm = small.tile([N, 1], F32, tag="zm")
                        nc.vector.tensor_reduce(out=zm, in_=tmp, axis=AX.X,
                                                op=OP.add)
                        ae = small.tile([N, 1], F32, tag="ae")
                        nc.vector.tensor_mul(ae, zm, rinv)
                        beta = small.tile([N, DIM], F32, tag="beta")
                        nc.gpsimd.tensor_scalar(out=beta, in0=u_cols,
                                                scalar1=ae, scalar2=None,
                                                op0=OP.mult)
                        nc.vector.tensor_add(beta, beta, w0)

                        # IN[i, d, j] = z[i,j]*s[d] + beta[i,d]
                        ine = inpool.tile([N, DIM, N], F32, tag="in")
                        for d in range(DIM):
                            if d not in dve_ds:
                                nc.gpsimd.tensor_scalar(
                                    out=ine[:, d, :], in0=ze,
                                    scalar1=s_cols[:, d:d + 1],
                                    scalar2=beta[:, d:d + 1],
                                    op0=OP.mult, op1=OP.add)
                            else:
                                nc.vector.scalar_tensor_tensor(
                                    out=ine[:, d, :], in0=ze,
                                    scalar=s_cols[:, d:d + 1],
                                    in1=_bcast_col(beta[:, d:d + 1], N),
                                    op0=OP.mult, op1=OP.add)

                        # gelu over the whole pair at once
                        nc.scalar.activation(out=gtile[:, :, k, :], in_=ine,
                                             func=AF.Gelu)

                    # reduce over d: psum[i,(k,j)] += w_d * G[i,d,(k,j)]
                    ps = psump.tile([N, 2 * N], F32, tag="ps")
                    for d in range(DIM):
                        nc.tensor.matmul(out=ps, lhsT=sd[:, d, :],
                                         rhs=gtile[:, d, :, :],
                                         start=(d == 0), stop=(d == DIM - 1))
                    t0 = (b * n_e + 2 * g) * N
                    nc.vector.tensor_scalar(
                        out=oall[:, t0:t0 + 2 * N], in0=ps,
                        scalar1=ob_col, scalar2=None, op0=OP.add)

            # global absmax over all outputs -> int8 scale for this core
            # (absmax = max(max(x), -min(x)); DVE abs_max reduce fails
            # walrus codegen)
            pmx = small.tile([N, 1], F32, tag="pmx")
            nc.vector.tensor_reduce(out=pmx, in_=oall, axis=AX.X, op=OP.max)
            pmn = small.tile([N, 1], F32, tag="pmn")
            nc.vector.tensor_reduce(out=pmn, in_=oall, axis=AX.X, op=OP.min)
            nc.vector.tensor_scalar_mul(pmn, pmn, -1.0)
            nc.vector.tensor_max(pmx, pmx, pmn)
            nc.vector.tensor_scalar_max(pmx, pmx, 1e-20)
            amax = singles.tile([N, 1], F32)
            nc.gpsimd.partition_all_reduce(amax, pmx, N, ReduceOp.absmax)
            invq = singles.tile([N, 1], F32)
            nc.vector.reciprocal(out=invq, in_=amax)
            nc.vector.tensor_scalar_mul(invq, invq, 127.0)
            sct = singles.tile([1, 1], F32)
            nc.vector.tensor_scalar_mul(sct, amax[0:1, :], 1.0 / 127.0)
            sc_dst = bass.AP(tensor=out_dram[0:1, 0:1].tensor,
                             offset=nt * N * N,
                             ap=[[4, 1], [1, 4]]).bitcast(F32)
            nc.sync.dma_start(out=sc_dst, in_=sct)

            # quantize: ONE ACT pass (out = Copy(in * invq) -> int8, RNE)
            oq8 = singles.tile([N, nt * N], I8)
            nc.scalar.activation(out=oq8, in_=oall, func=AF.Copy,
                                 scale=invq)
            for t in range(nt):
                nc.sync.dma_start(out=out_dram[t * N:(t + 1) * N, :],
                                  in_=oq8[:, t * N:(t + 1) * N])

    nc.compile()
    return nc


_RT = {}


def _build_runtime(dev_lo=0, dev_hi=NCORES, nc=None):
    """Build the Bass module once and wrap it in a cached AOT-compiled
    shard_map over devices[dev_lo:dev_hi]. Mirrors
    concourse.bass2jax.run_bass_via_pjrt, hoisting everything
    per-call-invariant (trace, lower, NEFF compile+load) out of kernel()."""
    ngrp = dev_hi - dev_lo
    if nc is None:
        nc = build_bass()
    b2j.install_neuronx_cc_hook()

    partition_name = (nc.partition_id_tensor.name
                      if nc.partition_id_tensor is not None else None)
    in_names, out_names, out_avals, in_specs = [], [], [], []
    for alloc in nc.m.functions[0].allocations:
        if not isinstance(alloc, mybir.MemoryLocationSet):
            continue
        name = alloc.memorylocations[0].name
        if alloc.kind == "ExternalInput":
            if name != partition_name:
                in_names.append(name)
                in_specs.append((tuple(alloc.tensor_shape),
                                 mybir.dt.np(alloc.dtype)))
        elif alloc.kind == "ExternalOutput":
            out_names.append(name)
            out_avals.append(jax.core.ShapedArray(
                tuple(alloc.tensor_shape), mybir.dt.np(alloc.dtype)))
    in_names_full = list(in_names)
    if partition_name is not None:
        in_names_full.append(partition_name)

    devices = jax.devices()[dev_lo:dev_hi]
    assert len(devices) == ngrp
    mesh = Mesh(np.asarray(devices), ("core",))
    out_avals_t = tuple(out_avals)
    in_names_t = tuple(in_names_full)
    out_names_t = tuple(out_names)

    def _body(*args):
        operands = list(args)
        if partition_name is not None:
            operands.append(b2j.partition_id_tensor())
        outs = b2j._bass_exec_p.bind(
            *operands,
            out_avals=out_avals_t,
            in_names=in_names_t,
            out_names=out_names_t,
            lowering_input_output_aliases=(),
            sim_require_finite=True,
            sim_require_nnan=True,
            nc=nc,
        )
        return tuple(outs)

    nin = len(in_names)
    jit_fn = jax.jit(
        shard_map(_body, mesh=mesh, in_specs=(PartitionSpec("core"),) * nin,
                  out_specs=(PartitionSpec("core"),) * len(out_names),
                  check_rep=False),
        keep_unused=True)

    shard = NamedSharding(mesh, PartitionSpec("core"))

    # AOT-compile with the bass effect suppressed: enables JAX's C++
    # fast-path dispatch and drops per-call effect-token ordering.
    in_sds = [jax.ShapeDtypeStruct((ngrp * s[0], *s[1:]), d, sharding=shard)
              for s, d in in_specs]
    try:
        fn = b2j.fast_dispatch_compile(lambda: jit_fn.lower(*in_sds).compile())
    except Exception:
        fn = jit_fn

    return dict(nc=nc, fn=fn, in_names=in_names, out_names=out_names,
                shard=shard, ngrp=ngrp)


def _get_runtimes():
    key = ("rt", N_B)
    if key not in _RT:
        nc = build_bass(ncores=NCORES_USED, n_b=N_B)
        _RT[key] = [_build_runtime(0, NCORES_USED, nc=nc)]
    return _RT[key]


def _consts_row(phi_w, phi_b, wself_w, wself_b, wctx_w, out_w, out_b):
    f = np.float32
    u = (wctx_w.astype(f) @ phi_w.astype(f)).astype(f)
    v = (wctx_w.astype(f) @ phi_b.astype(f)).astype(f)
    row = np.zeros((CC,), dtype=f)
    row[C_S:C_S + DIM] = wself_w.astype(f)
    row[C_U:C_U + DIM] = u
    row[C_V:C_V + DIM] = v
    row[C_WSB:C_WSB + DIM] = wself_b.astype(f)
    row[C_OB] = f(out_b)
    row[C_OW:C_OW + DIM] = out_w.astype(f)
    return row


_PACK = {}


def _pack_inputs(z_tilde, mask, crow, lo, hi):
    """Quantize z to int8 with a per-core absmax scale (each core handles
    N_B batches), pack the masks and the consts bytes into the same int8
    tensor, and fold the scale into per-core consts rows (the wself_w and
    u columns are pre-multiplied by it, so the on-chip math never sees
    the scale). Runs on the XLA CPU backend (vectorized)."""
    nb = hi - lo
    ncu = nb // N_B
    nt = N_B * E
    crows = (CC * 4 + N - 1) // N
    z = np.ascontiguousarray(z_tilde[lo:hi])
    m = np.ascontiguousarray(mask[lo:hi])
    # absmax in numpy: 10x faster than an XLA-CPU full reduce (0.5 vs 6 ms)
    zr = z.reshape(ncu, -1)
    sc = np.maximum(zr.max(axis=1), -zr.min(axis=1)).astype(np.float32)
    sc = np.maximum(sc / 127.0, np.float32(1e-30))
    # fold the z scale into the per-core consts row; its raw bytes ride
    # as the last rows of the packed tensor (bitcast back to f32 on-chip)
    c = np.tile(crow[None, :], (ncu, 1))
    c[:, C_S:C_S + DIM] *= sc[:, None]
    c[:, C_U:C_U + DIM] *= sc[:, None]
    key = ("pack", nb)
    if key not in _PACK:
        def _f(z, m, inv, c):
            z = z.reshape(ncu, N_B * E, N, N)
            q = jnp.clip(jnp.rint(z * inv[:, None, None, None]), -127, 127)
            q = q.astype(jnp.int8).reshape(ncu, nt * N, N)
            # bitpack masks MSB-first; linear layout: partition i's bits
            # at byte offset i*(N_B*16), batch b at [16b,16b+16)
            pw = jnp.array([128, 64, 32, 16, 8, 4, 2, 1], jnp.float32)
            mb = (m.reshape(ncu, N_B, N, 16, 8) * pw).sum(-1)
            mb = mb.astype(jnp.uint8).transpose(0, 2, 1, 3)
            mb = mb.reshape(ncu, N_B * 16, N)
            mb = jax.lax.bitcast_convert_type(mb, jnp.int8)
            cb = jax.lax.bitcast_convert_type(c, jnp.int8).reshape(ncu, -1)
            cb = jnp.pad(cb, ((0, 0), (0, crows * N - CC * 4)))
            return jnp.concatenate(
                [q, mb, cb.reshape(ncu, crows, N)], axis=1).reshape(
                    ncu * (nt * N + N_B * 16 + crows), N)
        _PACK[key] = jax.jit(_f, backend="cpu")
    return np.asarray(_PACK[key](z, m, (1.0 / sc), c))


def _args_for(rt, amap):
    args = []
    for name in rt["in_names"]:
        if name in amap:
            args.append(amap[name])
        else:
            # unexpected extra input (e.g. dbg tensor): zero-fill
            for alloc in rt["nc"].m.functions[0].allocations:
                if (isinstance(alloc, mybir.MemoryLocationSet)
                        and alloc.memorylocations[0].name == name):
                    shape = tuple(alloc.tensor_shape)
                    dt = mybir.dt.np(alloc.dtype)
                    args.append(
                        np.zeros((rt["ngrp"] * shape[0], *shape[1:]), dt))
                    break
            else:
                raise KeyError(name)
    return args


_DEQ = {}


def _dequant(oq, sc, nb):
    """int8 out [ncu, N_B, E, N, N] * per-core scale -> f32 [nb,E,N,N]."""
    key = ("deq", oq.shape)
    if key not in _DEQ:
        def _f(q, s):
            o = q.astype(jnp.float32) * s.reshape(-1, 1, 1, 1, 1)
            return o.reshape(nb, E, N, N)
        _DEQ[key] = jax.jit(_f, backend="cpu")
    return np.asarray(_DEQ[key](oq, sc))


def _unpack_out(arr, nb):
    """[ncu*(N_B*E*N+1), N] int8 wire tensor -> f32 [nb, E, N, N]."""
    ncu = nb // N_B
    nt = N_B * E
    arr = arr.reshape(ncu, nt * N + 1, N)
    sc = np.ascontiguousarray(arr[:, nt * N, 0:4]).view(np.float32)
    oq = arr[:, :nt * N, :].reshape(ncu, N_B, E, N, N)
    return _dequant(oq, sc.reshape(-1), nb)


def _run_pipelined(rt, z_tilde, mask, crow):
    """N_SPLIT sequential async dispatches of N_B batches each on core 0.
    Part p is packed right before its dispatch (pack overlaps the wire),
    uploads of part p overlap execute+download of part p-1, and every
    output is prefetched with copy_to_host_async before any blocking
    np.asarray (an un-prefetched fetch costs a full extra RTT, +80 ms).
    numpy args go straight into the compiled fn: its internal transfer
    beats an explicit device_put call (extra put calls serialize extra
    protocol phases, +60 ms)."""
    io = rt["out_names"].index("out")
    outs = []
    for p in range(N_SPLIT):
        packed = _pack_inputs(z_tilde, mask, crow,
                              p * N_B, (p + 1) * N_B)
        o = rt["fn"](*_args_for(rt, {"z": packed}))
        for x in o:
            x.copy_to_host_async()
        outs.append(o)
    return np.concatenate(
        [_unpack_out(np.asarray(o[io]), N_B) for o in outs], axis=0)


def _run_cached(z_tilde, mask, crow):
    rt = _get_runtimes()[0]
    out = _run_pipelined(rt, z_tilde, mask, crow)
    return np.ascontiguousarray(out, dtype=np.float32)


def _run_spmd(z_tilde, mask, crow):
    """Contract-faithful path through bass_utils.run_bass_kernel_spmd
    (used if the cached fast path fails; ~500 ms/part extra overhead)."""
    from concourse.bass_utils import run_bass_kernel_spmd

    nc = _get_runtimes()[0]["nc"]
    parts = []
    for p in range(N_SPLIT):
        packed = _pack_inputs(z_tilde, mask, crow,
                              p * N_B, (p + 1) * N_B)
        res = run_bass_kernel_spmd(nc, [{"z": packed}], [0])
        parts.append(_unpack_out(res.results[0]["out"], N_B))
    return np.ascontiguousarray(np.concatenate(parts, axis=0))


def _kernel_jax_fallback(z_tilde, mask, phi_w, phi_b, wself_w, wself_b,
                         wctx_w, out_w, out_b):
    """Pure-jax CPU fallback, used only if the Bass paths fail so the
    harness still gets a correct full output."""

    def one_batch(z, m):
        rowsum = m.sum(axis=1)
        denom = jnp.maximum(rowsum, 1.0)
        zm = jnp.einsum('eij,ij->ei', z, m)
        a = zm / denom
        r = rowsum / denom
        u = wctx_w.astype(np.float32) @ phi_w.astype(np.float32)
        v = wctx_w.astype(np.float32) @ phi_b.astype(np.float32)
        beta = (wself_b[None, None, :] + a[:, :, None] * u[None, None, :]
                + (r * 1.0)[None, :, None] * v[None, None, :])
        x = (z[..., None] * wself_w + beta[:, :, None, :])
        h = jax.nn.gelu(x, approximate=False)
        return jnp.einsum('eijd,d->eij', h, out_w) + out_b

    fn = jax.jit(one_batch, backend="cpu")
    outs = [np.asarray(fn(z_tilde[c], mask[c]))
            for c in range(z_tilde.shape[0])]
    return np.stack(outs, axis=0).astype(np.float32)


def kernel(**inputs):
    # normalize to host numpy (free for numpy inputs; one D2H if the
    # caller hands us jax arrays)
    inputs = {k: np.asarray(v) for k, v in inputs.items()}
    crow = _consts_row(
        inputs["phi_w"], inputs["phi_b"], inputs["wself_w"],
        inputs["wself_b"], inputs["wctx_w"], inputs["out_w"],
        inputs["out_b"])
    # transient device wedges (NRT_EXEC_UNIT_UNRECOVERABLE) usually clear
    # on retry — try the fast path twice before degrading
    for attempt in range(2):
        try:
            return _run_cached(inputs["z_tilde"], inputs["mask"], crow)
        except Exception:
            import traceback
            traceback.print_exc()
            if attempt == 0:
                import time
                time.sleep(2.0)
    try:
        return _run_spmd(inputs["z_tilde"], inputs["mask"], crow)
    except Exception:
        import traceback
        traceback.print_exc()
    return _kernel_jax_fallback(**inputs)



# revision 69
# speedup vs baseline: 1.2966x; 1.2966x over previous
"""Trainium2 Bass kernel for nn_DeepSetClassifier (deep-set pooling + gelu MLP).

Math (per batch b, expert e, row i, col j, hidden d; N=128, DIM=32):
    rowsum[i] = sum_j mask[i,j];  denom = max(rowsum, 1);  rinv = 1/denom
    zm[e,i]   = sum_j mask[i,j] * z[e,i,j]
    a[e,i] = zm*rinv ; r[i] = rowsum*rinv
    beta[e,i,d] = wself_b[d] + u[d]*a[e,i] + v[d]*r[i]     (u = wctx@phi_w, v = wctx@phi_b)
    out[e,i,j] = out_b + sum_d out_w[d] * gelu(wself_w[d]*z[e,i,j] + beta[e,i,d])

Sharding/dispatch: ALL work on ONE core, as N_SPLIT=4 pipelined async
dispatches of N_B=2 batches each. One core because the axon tunnel
charges a large per-device fan-out penalty (a 32-byte 8-way sharded
device_put costs ~85 ms; the same 1.25 MiB to one device costs ~72 ms)
while the extra on-chip work is <1 ms. Pipelined because part p's
upload overlaps part p-1's execute+download on the full-duplex wire,
and each part is packed right before its dispatch so host pack time
overlaps the wire too. Interleaved: 1 dispatch 93.5/95.2 ms (min/p25),
2x4-batch 83.6/89.0, 4x2-batch 77.2/84.0, 8x1-batch 85.6/90.1.

Engine plan per dispatch (N_B batches x 4 "pairs"; a pair = 2 e values):
  - DVE+GPSIMD: build IN[e][i,(d,j)] = z*s_d + beta_d
    (GPSIMD: fused tensor_scalar with two AP scalars — verified exact on HW.
     DVE: scalar_tensor_tensor with one AP scalar + broadcast tensor.)
  - ACT: one big gelu per pair over [128, 32*128]
  - PE: reduce over d via 32 accumulating matmuls with diagonal stationary
    w_d*I (float32r, moving N=256 = 2 pairs) into PSUM
  - DVE: PSUM + out_b -> fp16 SBUF (ulp << int8 step); after all pairs:
    global absmax (DVE max/min reduces + gpsimd partition_all_reduce),
    quantize with ONE fused ACT pass (Copy with scale=127/absmax; the
    ACT int8 output conversion rounds to nearest even — probed),
    DMA out + scale bytes

Dispatch plan (dominant cost — the 8 cores sit behind an axon tunnel with
~40-85 ms RTT, highly variable latency, and upstream-expensive wire):
  - The jitted shard_map executable is built ONCE per process and cached;
    re-running run_bass_kernel_spmd per call re-traces, re-lowers and
    re-loads the NEFF (~500 ms/call).
  - AOT-compiled with the bass effect suppressed (C++ fast-path dispatch).
  - sdiag (the 2 MiB/core PE stationary w_d*I) is built on-chip with one
    gpsimd affine_select over an iota predicate instead of shipped (16 MiB).
  - No donated zero output buffers: outputs are plain custom-call results
    (the kernel writes every element).
  - Wire format up, per part: ONE tightly packed [2182,128] int8 tensor
    (273 KiB): rows 0..2047 = the part's 16 z matrices quantized int8
    (one absmax scale per part, folded into the host-side consts:
    wself_w and u pre-multiplied, so the on-chip math never sees the
    scale); rows 2048..2175 = the part's masks BITPACKED (row i, cols
    [16b,16b+16) = packbits(mask[b,i,:]); unpacked on-chip with DVE
    shift+and, probed exact); rows 2176.. = the 161 f32 consts as raw
    bytes (bitcast back to f32 on-chip).
  - Wire format down, per part: ONE [2049,128] int8 tensor (256 KiB):
    rows 0..2047 = out int8 (scale = on-chip absmax via DVE max/min
    reduces + gpsimd partition_all_reduce), row 2048 carries the f32
    scale bytes. Dequantized on host. Combined z-int8 + out-int8 + fp16
    out staging rel err 1.09% (measured = simulated) vs the 2e-2 gate.
  - ONE input tensor and ONE output tensor per part, passed as numpy
    args to the AOT-compiled fn: every extra put call serializes an
    additional protocol phase on the tunnel (measured +60 ms), and every
    un-prefetched output fetch costs a full extra RTT (measured +80 ms)
    — hence copy_to_host_async on every output right at dispatch.
  - Net: ~77-95 ms/call at a ~72-85 ms pure-RTT floor (tunnel drifts;
    the 8-core fp16 predecessor measured 94-148 ms, same-window 148->85).
"""

import numpy as np

import jax
import jax.numpy as jnp
from jax.experimental.shard_map import shard_map
from jax.sharding import Mesh, NamedSharding, PartitionSpec

import concourse.bass as bass
import concourse.bacc as bacc
import concourse.tile as tile
from concourse import mybir
from concourse import bass2jax as b2j
from concourse.bass_isa import ReduceOp

F32 = mybir.dt.float32
F32R = mybir.dt.float32r
HALF = mybir.dt.float16
U8 = mybir.dt.uint8
I8 = mybir.dt.int8
AX = mybir.AxisListType
OP = mybir.AluOpType
AF = mybir.ActivationFunctionType

E, N, DIM = 8, 128, 32
NCORES = 8
# All work runs on ONE core: the axon tunnel charges a large per-device
# fan-out penalty (a 32-byte 8-way sharded device_put costs ~85 ms; the
# SAME 1.25 MiB to one device costs ~72 ms), while the extra on-chip
# work is <1 ms. Measured min 96.4 (1 core) vs 102.7 ms (8 cores).
# The call is further PIPELINED as N_SPLIT sequential dispatches of N_B
# batches each on that core: part p's upload overlaps part p-1's
# execute+download on the full-duplex tunnel, and each part is packed
# right before its dispatch so host pack time overlaps the wire too.
# Interleaved: full 93.5/95.2 (min/p25), 2x4-batch 83.6/89.0,
# 4x2-batch 77.2/84.0, 8x1-batch 85.6/90.1 -> 4 parts of 2 batches.
N_B = 2
N_SPLIT = 4
NCORES_USED = 1

# consts layout (columns of the [1, CC] consts input; broadcast down
# partitions on-chip): wself_w | u | v | wself_b | out_b | out_w
C_S = 0
C_U = DIM
C_V = 2 * DIM
C_WSB = 3 * DIM
C_OB = 4 * DIM
C_OW = 4 * DIM + 1
CC = 5 * DIM + 1

PE_DTYPE = F32R
N_DVE_DS = 16

def _bcast_col(col_ap, n):
    """[128,1] column AP -> [128,n] stride-0 broadcast along free dim."""
    return bass.AP(tensor=col_ap.tensor, offset=col_ap.offset,
                   ap=[col_ap.ap[0], [0, n]])


def _ow_diag_src(consts, n):
    """AP reading consts[i, C_OW+d] at logical index [i, d, j] (j bcast)."""
    base = consts[:, C_OW:C_OW + DIM]
    return bass.AP(tensor=base.tensor, offset=base.offset,
                   ap=[base.ap[0], list(base.ap[1]), [0, n]])


def build_bass(ncores=None, n_e=E, n_b=None):
    """n_b = batches handled by EACH core (1 = classic 8-core data
    parallel; 8 = the whole problem on one core, which avoids the
    per-device fan-out penalty of the axon tunnel)."""
    pe_dt = PE_DTYPE
    if n_b is None:
        n_b = N_B
    nc = bacc.Bacc("TRN2", target_bir_lowering=False, debug=False,
                   num_devices=ncores or (NCORES // n_b))
    nt = n_b * n_e

    # ONE input tensor, tightly packed 2D [R, N] int8:
    #   rows [t*N,(t+1)*N) for t=b*n_e+e : z[b,e] quantized int8 (absmax
    #     scale over the core's batches, folded into the host-side consts)
    #   next n_b*16 rows : mask BITS, linear layout — partition i's bits
    #     for all n_b batches at byte offset i*(n_b*16), batch b at
    #     [16b,16b+16), MSB-first packbits(mask[b,i,:]) (2 KiB/batch
    #     instead of 16)
    #   last rows : the CC f32 consts as raw bytes (bitcast on read).
    # One tensor = one transfer per dispatch on the tunnel.
    mrows = n_b * 16
    zrows = nt * N + mrows + (CC * 4 + N - 1) // N
    z_dram = nc.dram_tensor("z", [zrows, N], I8, kind="ExternalInput")
    # ONE output tensor: rows [(b*n_e+e)*N + i] = out int8; row [nt*N]
    # bytes 0..3 = the on-chip-computed absmax scale as raw f32 bytes.
    # The f32->int8 ACT output conversion rounds to nearest even (probed).
    out_dram = nc.dram_tensor("out", [nt * N + 1, N], I8,
                              kind="ExternalOutput")

    dve_ds = tuple(range(N_DVE_DS))

    with tile.TileContext(nc) as tc:
        with (
            tc.tile_pool(name="singles", bufs=1) as singles,
            tc.tile_pool(name="zpool", bufs=4) as zpool,
            tc.tile_pool(name="small", bufs=4) as small,
            tc.tile_pool(name="mpool", bufs=2) as mpool,
            tc.tile_pool(name="inpool", bufs=3) as inpool,
            tc.tile_pool(name="gpool", bufs=2) as gpool,
            tc.tile_pool(name="psum", bufs=3, space="PSUM") as psump,
        ):
            # bcast [1,CC] consts row down 128 partitions: ones^T @ row
            crow = singles.tile([1, CC], F32)
            c_src = bass.AP(tensor=z_dram[0:1, 0:1].tensor,
                            offset=(nt * N + mrows) * N,
                            ap=[[4, 1], [1, CC * 4]]).bitcast(F32)
            nc.sync.dma_start(out=crow, in_=c_src)

            # ONE DMA pulls every batch's mask bits: partition i <- the
            # n_b*16 bytes at linear offset i*(n_b*16) of the bits block
            mball = singles.tile([N, mrows], U8)
            nc.sync.dma_start(
                out=mball,
                in_=bass.AP(tensor=z_dram[0:1, 0:1].tensor,
                            offset=nt * N * N,
                            ap=[[mrows, N], [1, mrows]]).bitcast(U8))
            ones = singles.tile([1, N], F32)
            nc.gpsimd.memset(ones, 1.0)
            ps_c = psump.tile([N, CC], F32, tag="cbcast")
            nc.tensor.matmul(out=ps_c, lhsT=ones, rhs=crow,
                             start=True, stop=True)
            consts = singles.tile([N, CC], F32)
            nc.scalar.copy(out=consts, in_=ps_c)

            s_cols = consts[:, C_S:C_S + DIM]
            u_cols = consts[:, C_U:C_U + DIM]
            v_cols = consts[:, C_V:C_V + DIM]
            wsb_cols = consts[:, C_WSB:C_WSB + DIM]
            ob_col = consts[:, C_OB:C_OB + 1]

            # sd[i, d, j] = out_w[d] * (i == j) — PE stationary, built on-chip
            sd = singles.tile([N, DIM, N], pe_dt)
            nc.gpsimd.affine_select(
                out=sd[:, :, :], in_=_ow_diag_src(consts, N),
                pattern=[[0, DIM], [-1, N]], compare_op=OP.is_equal,
                fill=0.0, base=0, channel_multiplier=1)

            # all outputs stay on-chip (fp16: ulp << the int8 step) until
            # the global absmax is known, then quantize in one ACT pass
            oall = singles.tile([N, nt * N], HALF)

            for b in range(n_b):
                # --- mask pooling prep (once per batch) ---
                # unpack mask bits: msk[i, 8k+t] = (byte[i,k] >> (7-t)) & 1
                mb8 = mball[:, 16 * b:16 * b + 16]
                mu8 = mpool.tile([N, N], U8, tag="mu8")
                for t in range(8):
                    view = bass.AP(tensor=mu8.tensor, offset=mu8.offset + t,
                                   ap=[mu8.ap[0], [8, 16]])
                    nc.vector.tensor_scalar(
                        out=view, in0=mb8, scalar1=7 - t, scalar2=1,
                        op0=OP.logical_shift_right, op1=OP.bitwise_and)
                msk = mpool.tile([N, N], F32, tag="m")
                nc.scalar.copy(out=msk, in_=mu8)
                rowsum = small.tile([N, 1], F32, tag="rowsum")
                nc.vector.tensor_reduce(out=rowsum, in_=msk, axis=AX.X,
                                        op=OP.add)
                denom = small.tile([N, 1], F32, tag="denom")
                nc.vector.tensor_scalar_max(denom, rowsum, 1.0)
                rinv = small.tile([N, 1], F32, tag="rinv")
                nc.vector.reciprocal(out=rinv, in_=denom)
                rr = small.tile([N, 1], F32, tag="rr")
                nc.vector.tensor_mul(rr, rowsum, rinv)
                # W0[i,d] = wself_b[d] + v[d]*r[i] (gpsimd fused 2-op ok)
                w0 = small.tile([N, DIM], F32, tag="w0")
                nc.gpsimd.tensor_scalar(out=w0, in0=v_cols, scalar1=rr,
                                        scalar2=None, op0=OP.mult)
                nc.vector.tensor_add(w0, w0, wsb_cols)

                for g in range(n_e // 2):
                    gtile = gpool.tile([N, DIM, 2, N], pe_dt, tag="g2")
                    for k in range(2):
                        e = 2 * g + k
                        t0z = (b * n_e + e) * N
                        ze_raw = zpool.tile([N, N], I8, tag="zraw")
                        nc.sync.dma_start(out=ze_raw,
                                          in_=z_dram[t0z:t0z + N, :])
                        ze = zpool.tile([N, N], F32, tag="z")
                        nc.scalar.copy(out=ze, in_=ze_raw)

                        # zm[i] = sum_j mask*z
                        tmp = zpool.tile([N, N], F32, tag="tmp")
                        nc.vector.tensor_mul(tmp, ze, msk)
                        zm = small.tile([N, 1], F32, tag="zm")
                        nc.vector.tensor_reduce(out=zm, in_=tmp, axis=AX.X,
                                                op=OP.add)
                        ae = small.tile([N, 1], F32, tag="ae")
                        nc.vector.tensor_mul(ae, zm, rinv)
                        beta = small.tile([N, DIM], F32, tag="beta")
                        nc.gpsimd.tensor_scalar(out=beta, in0=u_cols,
                                                scalar1=ae, scalar2=None,
                                                op0=OP.mult)
                        nc.vector.tensor_add(beta, beta, w0)

                        # IN[i, d, j] = z[i,j]*s[d] + beta[i,d]
                        ine = inpool.tile([N, DIM, N], F32, tag="in")
                        for d in range(DIM):
                            if d not in dve_ds:
                                nc.gpsimd.tensor_scalar(
                                    out=ine[:, d, :], in0=ze,
                                    scalar1=s_cols[:, d:d + 1],
                                    scalar2=beta[:, d:d + 1],
                                    op0=OP.mult, op1=OP.add)
                            else:
                                nc.vector.scalar_tensor_tensor(
                                    out=ine[:, d, :], in0=ze,
                                    scalar=s_cols[:, d:d + 1],
                                    in1=_bcast_col(beta[:, d:d + 1], N),
                                    op0=OP.mult, op1=OP.add)

                        # gelu over the whole pair at once
                        nc.scalar.activation(out=gtile[:, :, k, :], in_=ine,
                                             func=AF.Gelu)

                    # reduce over d: psum[i,(k,j)] += w_d * G[i,d,(k,j)]
                    ps = psump.tile([N, 2 * N], F32, tag="ps")
                    for d in range(DIM):
                        nc.tensor.matmul(out=ps, lhsT=sd[:, d, :],
                                         rhs=gtile[:, d, :, :],
                                         start=(d == 0), stop=(d == DIM - 1))
                    t0 = (b * n_e + 2 * g) * N
                    nc.vector.tensor_scalar(
                        out=oall[:, t0:t0 + 2 * N], in0=ps,
                        scalar1=ob_col, scalar2=None, op0=OP.add)

            # global absmax over all outputs -> int8 scale for this core
            # (absmax = max(max(x), -min(x)); DVE abs_max reduce fails
            # walrus codegen)
            pmx = small.tile([N, 1], F32, tag="pmx")
            nc.vector.tensor_reduce(out=pmx, in_=oall, axis=AX.X, op=OP.max)
            pmn = small.tile([N, 1], F32, tag="pmn")
            nc.vector.tensor_reduce(out=pmn, in_=oall, axis=AX.X, op=OP.min)
            nc.vector.tensor_scalar_mul(pmn, pmn, -1.0)
            nc.vector.tensor_max(pmx, pmx, pmn)
            nc.vector.tensor_scalar_max(pmx, pmx, 1e-20)
            amax = singles.tile([N, 1], F32)
            nc.gpsimd.partition_all_reduce(amax, pmx, N, ReduceOp.absmax)
            invq = singles.tile([N, 1], F32)
            nc.vector.reciprocal(out=invq, in_=amax)
            nc.vector.tensor_scalar_mul(invq, invq, 127.0)
            sct = singles.tile([1, 1], F32)
            nc.vector.tensor_scalar_mul(sct, amax[0:1, :], 1.0 / 127.0)
            sc_dst = bass.AP(tensor=out_dram[0:1, 0:1].tensor,
                             offset=nt * N * N,
                             ap=[[4, 1], [1, 4]]).bitcast(F32)
            nc.sync.dma_start(out=sc_dst, in_=sct)

            # quantize: ONE ACT pass (out = Copy(in * invq) -> int8, RNE)
            oq8 = singles.tile([N, nt * N], I8)
            nc.scalar.activation(out=oq8, in_=oall, func=AF.Copy,
                                 scale=invq)
            for t in range(nt):
                nc.sync.dma_start(out=out_dram[t * N:(t + 1) * N, :],
                                  in_=oq8[:, t * N:(t + 1) * N])

    nc.compile()
    return nc


_RT = {}


def _build_runtime(dev_lo=0, dev_hi=NCORES, nc=None):
    """Build the Bass module once and wrap it in a cached AOT-compiled
    shard_map over devices[dev_lo:dev_hi]. Mirrors
    concourse.bass2jax.run_bass_via_pjrt, hoisting everything
    per-call-invariant (trace, lower, NEFF compile+load) out of kernel()."""
    ngrp = dev_hi - dev_lo
    if nc is None:
        nc = build_bass()
    b2j.install_neuronx_cc_hook()

    partition_name = (nc.partition_id_tensor.name
                      if nc.partition_id_tensor is not None else None)
    in_names, out_names, out_avals, in_specs = [], [], [], []
    for alloc in nc.m.functions[0].allocations:
        if not isinstance(alloc, mybir.MemoryLocationSet):
            continue
        name = alloc.memorylocations[0].name
        if alloc.kind == "ExternalInput":
            if name != partition_name:
                in_names.append(name)
                in_specs.append((tuple(alloc.tensor_shape),
                                 mybir.dt.np(alloc.dtype)))
        elif alloc.kind == "ExternalOutput":
            out_names.append(name)
            out_avals.append(jax.core.ShapedArray(
                tuple(alloc.tensor_shape), mybir.dt.np(alloc.dtype)))
    in_names_full = list(in_names)
    if partition_name is not None:
        in_names_full.append(partition_name)

    devices = jax.devices()[dev_lo:dev_hi]
    assert len(devices) == ngrp
    mesh = Mesh(np.asarray(devices), ("core",))
    out_avals_t = tuple(out_avals)
    in_names_t = tuple(in_names_full)
    out_names_t = tuple(out_names)

    def _body(*args):
        operands = list(args)
        if partition_name is not None:
            operands.append(b2j.partition_id_tensor())
        outs = b2j._bass_exec_p.bind(
            *operands,
            out_avals=out_avals_t,
            in_names=in_names_t,
            out_names=out_names_t,
            lowering_input_output_aliases=(),
            sim_require_finite=True,
            sim_require_nnan=True,
            nc=nc,
        )
        return tuple(outs)

    nin = len(in_names)
    jit_fn = jax.jit(
        shard_map(_body, mesh=mesh, in_specs=(PartitionSpec("core"),) * nin,
                  out_specs=(PartitionSpec("core"),) * len(out_names),
                  check_rep=False),
        keep_unused=True)

    shard = NamedSharding(mesh, PartitionSpec("core"))

    # AOT-compile with the bass effect suppressed: enables JAX's C++
    # fast-path dispatch and drops per-call effect-token ordering.
    in_sds = [jax.ShapeDtypeStruct((ngrp * s[0], *s[1:]), d, sharding=shard)
              for s, d in in_specs]
    try:
        fn = b2j.fast_dispatch_compile(lambda: jit_fn.lower(*in_sds).compile())
    except Exception:
        fn = jit_fn

    return dict(nc=nc, fn=fn, in_names=in_names, out_names=out_names,
                shard=shard, ngrp=ngrp)


def _get_runtimes():
    key = ("rt", N_B)
    if key not in _RT:
        nc = build_bass(ncores=NCORES_USED, n_b=N_B)
        _RT[key] = [_build_runtime(0, NCORES_USED, nc=nc)]
    return _RT[key]


def _consts_row(phi_w, phi_b, wself_w, wself_b, wctx_w, out_w, out_b):
    f = np.float32
    u = (wctx_w.astype(f) @ phi_w.astype(f)).astype(f)
    v = (wctx_w.astype(f) @ phi_b.astype(f)).astype(f)
    row = np.zeros((CC,), dtype=f)
    row[C_S:C_S + DIM] = wself_w.astype(f)
    row[C_U:C_U + DIM] = u
    row[C_V:C_V + DIM] = v
    row[C_WSB:C_WSB + DIM] = wself_b.astype(f)
    row[C_OB] = f(out_b)
    row[C_OW:C_OW + DIM] = out_w.astype(f)
    return row


_PACK = {}


def _pack_inputs(z_tilde, mask, crow, lo, hi, n_b=None):
    """Quantize batches [lo:hi] to int8 with an absmax scale per n_b-batch
    group, pack the masks and the consts bytes into the same int8 tensor,
    and fold the scale into per-group consts rows (the wself_w and u
    columns are pre-multiplied by it, so the on-chip math never sees the
    scale). Runs on the XLA CPU backend (vectorized)."""
    if n_b is None:
        n_b = N_B
    nb = hi - lo
    ncu = nb // n_b
    nt = n_b * E
    crows = (CC * 4 + N - 1) // N
    z = np.ascontiguousarray(z_tilde[lo:hi])
    m = np.ascontiguousarray(mask[lo:hi])
    # absmax in numpy: 10x faster than an XLA-CPU full reduce (0.5 vs 6 ms)
    zr = z.reshape(ncu, -1)
    sc = np.maximum(zr.max(axis=1), -zr.min(axis=1)).astype(np.float32)
    sc = np.maximum(sc / 127.0, np.float32(1e-30))
    # fold the z scale into the per-group consts row; its raw bytes ride
    # as the last rows of the packed tensor (bitcast back to f32 on-chip)
    c = np.tile(crow[None, :], (ncu, 1))
    c[:, C_S:C_S + DIM] *= sc[:, None]
    c[:, C_U:C_U + DIM] *= sc[:, None]
    key = ("pack", nb, n_b)
    if key not in _PACK:
        def _f(z, m, inv, c):
            z = z.reshape(ncu, n_b * E, N, N)
            q = jnp.clip(jnp.rint(z * inv[:, None, None, None]), -127, 127)
            q = q.astype(jnp.int8).reshape(ncu, nt * N, N)
            # bitpack masks MSB-first; linear layout: partition i's bits
            # at byte offset i*(n_b*16), batch b at [16b,16b+16)
            pw = jnp.array([128, 64, 32, 16, 8, 4, 2, 1], jnp.float32)
            mb = (m.reshape(ncu, n_b, N, 16, 8) * pw).sum(-1)
            mb = mb.astype(jnp.uint8).transpose(0, 2, 1, 3)
            mb = mb.reshape(ncu, n_b * 16, N)
            mb = jax.lax.bitcast_convert_type(mb, jnp.int8)
            cb = jax.lax.bitcast_convert_type(c, jnp.int8).reshape(ncu, -1)
            cb = jnp.pad(cb, ((0, 0), (0, crows * N - CC * 4)))
            return jnp.concatenate(
                [q, mb, cb.reshape(ncu, crows, N)], axis=1).reshape(
                    ncu * (nt * N + n_b * 16 + crows), N)
        _PACK[key] = jax.jit(_f, backend="cpu")
    return np.asarray(_PACK[key](z, m, (1.0 / sc), c))


def _args_for(rt, amap):
    args = []
    for name in rt["in_names"]:
        if name in amap:
            args.append(amap[name])
        else:
            # unexpected extra input (e.g. dbg tensor): zero-fill
            for alloc in rt["nc"].m.functions[0].allocations:
                if (isinstance(alloc, mybir.MemoryLocationSet)
                        and alloc.memorylocations[0].name == name):
                    shape = tuple(alloc.tensor_shape)
                    dt = mybir.dt.np(alloc.dtype)
                    args.append(
                        np.zeros((rt["ngrp"] * shape[0], *shape[1:]), dt))
                    break
            else:
                raise KeyError(name)
    return args


_DEQ = {}


def _dequant(oq, sc, nb):
    """int8 out [ncu, N_B, E, N, N] * per-core scale -> f32 [nb,E,N,N]."""
    key = ("deq", oq.shape)
    if key not in _DEQ:
        def _f(q, s):
            o = q.astype(jnp.float32) * s.reshape(-1, 1, 1, 1, 1)
            return o.reshape(nb, E, N, N)
        _DEQ[key] = jax.jit(_f, backend="cpu")
    return np.asarray(_DEQ[key](oq, sc))


def _unpack_out(arr, nb, n_b=None):
    """[ncu*(n_b*E*N+1), N] int8 wire tensor -> f32 [nb, E, N, N]."""
    if n_b is None:
        n_b = N_B
    ncu = nb // n_b
    nt = n_b * E
    arr = arr.reshape(ncu, nt * N + 1, N)
    sc = np.ascontiguousarray(arr[:, nt * N, 0:4]).view(np.float32)
    oq = arr[:, :nt * N, :].reshape(ncu, n_b, E, N, N)
    return _dequant(oq, sc.reshape(-1), nb)


def _run_pipelined(rt, z_tilde, mask, crow):
    """N_SPLIT sequential async dispatches of N_B batches each on core 0.
    Part p is packed right before its dispatch (pack overlaps the wire),
    uploads of part p overlap execute+download of part p-1, and every
    output is prefetched with copy_to_host_async before any blocking
    np.asarray (an un-prefetched fetch costs a full extra RTT, +80 ms).
    numpy args go straight into the compiled fn: its internal transfer
    beats an explicit device_put call (extra put calls serialize extra
    protocol phases, +60 ms)."""
    io = rt["out_names"].index("out")
    outs = []
    for p in range(N_SPLIT):
        packed = _pack_inputs(z_tilde, mask, crow,
                              p * N_B, (p + 1) * N_B)
        o = rt["fn"](*_args_for(rt, {"z": packed}))
        for x in o:
            x.copy_to_host_async()
        outs.append(o)
    return np.concatenate(
        [_unpack_out(np.asarray(o[io]), N_B) for o in outs], axis=0)


def _run_cached(z_tilde, mask, crow):
    rt = _get_runtimes()[0]
    out = _run_pipelined(rt, z_tilde, mask, crow)
    return np.ascontiguousarray(out, dtype=np.float32)


def _run_spmd(z_tilde, mask, crow):
    """Contract-faithful path through bass_utils.run_bass_kernel_spmd
    (used if the cached fast path fails; ~500 ms/part extra overhead)."""
    from concourse.bass_utils import run_bass_kernel_spmd

    nc = _get_runtimes()[0]["nc"]
    parts = []
    for p in range(N_SPLIT):
        packed = _pack_inputs(z_tilde, mask, crow,
                              p * N_B, (p + 1) * N_B)
        res = run_bass_kernel_spmd(nc, [{"z": packed}], [0])
        parts.append(_unpack_out(res.results[0]["out"], N_B))
    return np.ascontiguousarray(np.concatenate(parts, axis=0))


def _kernel_jax_fallback(z_tilde, mask, phi_w, phi_b, wself_w, wself_b,
                         wctx_w, out_w, out_b):
    """Pure-jax CPU fallback, used only if the Bass paths fail so the
    harness still gets a correct full output."""

    def one_batch(z, m):
        rowsum = m.sum(axis=1)
        denom = jnp.maximum(rowsum, 1.0)
        zm = jnp.einsum('eij,ij->ei', z, m)
        a = zm / denom
        r = rowsum / denom
        u = wctx_w.astype(np.float32) @ phi_w.astype(np.float32)
        v = wctx_w.astype(np.float32) @ phi_b.astype(np.float32)
        beta = (wself_b[None, None, :] + a[:, :, None] * u[None, None, :]
                + (r * 1.0)[None, :, None] * v[None, None, :])
        x = (z[..., None] * wself_w + beta[:, :, None, :])
        h = jax.nn.gelu(x, approximate=False)
        return jnp.einsum('eijd,d->eij', h, out_w) + out_b

    fn = jax.jit(one_batch, backend="cpu")
    outs = [np.asarray(fn(z_tilde[c], mask[c]))
            for c in range(z_tilde.shape[0])]
    return np.stack(outs, axis=0).astype(np.float32)


def kernel(**inputs):
    # normalize to host numpy (free for numpy inputs; one D2H if the
    # caller hands us jax arrays)
    inputs = {k: np.asarray(v) for k, v in inputs.items()}
    crow = _consts_row(
        inputs["phi_w"], inputs["phi_b"], inputs["wself_w"],
        inputs["wself_b"], inputs["wctx_w"], inputs["out_w"],
        inputs["out_b"])
    # transient device wedges (NRT_EXEC_UNIT_UNRECOVERABLE) usually clear
    # on retry — try the fast path twice before degrading
    for attempt in range(2):
        try:
            return _run_cached(inputs["z_tilde"], inputs["mask"], crow)
        except Exception:
            import traceback
            traceback.print_exc()
            if attempt == 0:
                import time
                time.sleep(2.0)
    try:
        return _run_spmd(inputs["z_tilde"], inputs["mask"], crow)
    except Exception:
        import traceback
        traceback.print_exc()
    return _kernel_jax_fallback(**inputs)

